# revision 1
# baseline (speedup 1.0000x reference)
"""Depthwise causal Conv1d (K=16) for x:(4, 2048, 8192) f32 on 8 TRN2 NeuronCores.

Strategy (tensor-parallel over channels, no cross-core communication):
  - Each core owns 256 channels (2048 / 8) for all 4 batches.
  - The time axis is cut into overlapping 79-sample windows with stride 64
    (15-sample causal halo), placed on SBUF partitions 0..78 and
    time-REVERSED within each window.  The depthwise conv of one channel is
    a banded-Toeplitz matmul on the TensorEngine:
        psum[m, (b,j)] = sum_p A[p, m] * X[p, (b,j)]
        A[p, m]     = w[78 - p - m]             for 63 <= p + m <= 78
        X[p, (b,j)] = x[b, c, 64*j + 63 - p]    (zero outside [0, T))
        psum[m, (b,j)] = y[b, c, 64*j + m]
    PO=64 minimizes total HBM bytes: the A-band costs (PO+15)*PO/32768 and
    the x halo costs (PO+15)/PO elems per output; the sum is minimal near
    PO=64, and 64 divides T=8192 exactly (no tail waste).
  - Because PO=64 = half the PE array, channel PAIRS run CONCURRENTLY via
    column tiling: even channel -> array cols [0:64), PSUM partitions
    [0:64); odd channel -> cols [64:128) via tile_position=(0,64), PSUM
    partitions [64:128).  PSUM drains then engage all 128 partitions.
  - Everything is bf16 on the wire (rel-err ~3e-3, gate 2e-2); PSUM
    accumulates in f32; the PSUM->SBUF drain downcasts to bf16.
  - Bias is added on the host (it is identically zero in this problem).

The host does the sharding + window-layout transposes with numpy; the device
kernel sees only dense p-major arrays.
"""

import os
import sys

import ml_dtypes
import numpy as np
from numpy.lib.stride_tricks import sliding_window_view

if "/opt/trn_rl_repo" not in sys.path:
    sys.path.insert(0, "/opt/trn_rl_repo")

import concourse.bacc as bacc
import concourse.mybir as mybir
import concourse.tile as tile
from concourse.bass_utils import run_bass_kernel_spmd

F32 = mybir.dt.float32
BF16 = mybir.dt.bfloat16
NP_BF16 = np.dtype(ml_dtypes.bfloat16)
ACT_COPY = mybir.ActivationFunctionType.Copy

N_CORES = 8
B = 4             # batch
DIM = 2048        # channels
T = 8192          # time
K = 16            # conv taps
C = DIM // N_CORES    # channels per core = 256
PO = 64           # outputs per window
PIN = PO + K - 1  # matmul contraction rows = 79
NJ = T // PO      # windows per (batch, channel) = 128 (exact)
XC = B * NJ       # x / out cols per channel = 512
CH = 8            # channels per device chunk
NCHUNK = C // CH  # 32
AGRP = 8          # chunks per A-load group
NAGRP = NCHUNK // AGRP  # 4
# DRAM row-stride padding: without it the partition strides of xin/yout/a_in
# are exact powers of two (256KB/128KB/32KB) and HBM bank aliasing collapses
# DMA throughput ~3x.  Padding the channel dims breaks the alignment while
# keeping each chunk's per-partition run contiguous; pad bytes never move.
XPAD = 16         # xin channels pad: partition stride (256+16)*512*2 = 272KB
YPAD = 4          # yout c2 pad: m-stride (128+4)*512*2 = 132KB
APAD = 8          # a_in channels pad: partition stride (256+8)*64*2 = 33KB


_compiled_nc = None


def _build_kernel():
    nc = bacc.Bacc(None)

    xin = nc.declare_dram_parameter("xin", [PIN, C + XPAD, XC], BF16, isOutput=False)
    a_in = nc.declare_dram_parameter("a_in", [PIN, C + APAD, PO], BF16, isOutput=False)
    # yout[o, m, c2, col]: channel = 2*c2 + o; col = (b, j); t = 64*j + m
    yout = nc.declare_dram_parameter(
        "yout", [2, PO, C // 2 + YPAD, XC], BF16, isOutput=True
    )

    ablate = os.environ.get("CONV_ABLATE", "") == "dmaonly"

    with tile.TileContext(nc) as tc:
        with (
            tc.tile_pool(name="xpool", bufs=6) as xpool,
            tc.tile_pool(name="apool", bufs=2) as apool,
            tc.tile_pool(name="opool", bufs=6) as opool,
            tc.tile_pool(name="psum", bufs=4, space="PSUM") as pspool,
        ):
            for g in range(NAGRP):
                a_t = apool.tile([PIN, AGRP * CH * PO], BF16)
                a_eng = nc.scalar if g % 2 == 0 else nc.sync
                a_eng.dma_start(
                    out=a_t[:].rearrange("p (c m) -> p c m", m=PO),
                    in_=a_in[:, g * AGRP * CH : (g + 1) * AGRP * CH, :],
                )
                for cc in range(AGRP):
                    chunk = g * AGRP + cc
                    c0 = chunk * CH
                    x_t = xpool.tile([PIN, CH * XC], BF16)
                    o_t = opool.tile([128, (CH // 2) * XC], BF16)

                    nc.gpsimd.dma_start(
                        out=x_t[:].rearrange("p (c j) -> p c j", c=CH),
                        in_=xin[:, c0 : c0 + CH, :],
                    )

                    if ablate:
                        nc.vector.tensor_copy(o_t[0:PIN, 0:XC], x_t[:, 0:XC])
                    else:
                        for t in range(CH // 4):
                            ps = pspool.tile([128, 2 * XC], F32)
                            for h in range(4):
                                i = 4 * t + h
                                pb = 64 * (h % 2)       # psum partition base
                                cb = XC * (h // 2)      # psum column base
                                nc.tensor.matmul(
                                    ps[pb : pb + PO, cb : cb + XC],
                                    a_t[:, (cc * CH + i) * PO : (cc * CH + i + 1) * PO],
                                    x_t[:, i * XC : (i + 1) * XC],
                                    start=True,
                                    stop=True,
                                    tile_position=(0, pb),
                                )
                            dst = o_t[:, t * 2 * XC : (t + 1) * 2 * XC]
                            if (chunk + t) % 2 == 0:
                                nc.vector.tensor_copy(dst, ps[:])
                            else:
                                nc.scalar.activation(dst, ps[:], ACT_COPY)

                    for o in range(2):
                        s_eng = nc.sync if (chunk + o) % 2 == 0 else nc.scalar
                        s_eng.dma_start(
                            out=yout[
                                o, :, (CH // 2) * chunk : (CH // 2) * (chunk + 1), :
                            ],
                            in_=o_t[64 * o : 64 * o + 64, :],
                        )

    nc.compile()
    return nc


def _get_nc():
    global _compiled_nc
    if _compiled_nc is None:
        _compiled_nc = _build_kernel()
    return _compiled_nc


def _prep_core(x, weight, core):
    """Build the per-core input map (numpy only)."""
    cs = slice(core * C, (core + 1) * C)
    xs = x[:, cs, :]                       # [B, C, T]
    w = weight[cs, 0, :]                   # [C, K]

    # X[p, c, (b, j)] = xpad[b, c, 64*j + 78 - p]; xpad = [15 zeros] ++ x
    xpad = np.zeros((B, C, K - 1 + T), dtype=np.float32)
    xpad[:, :, K - 1 :] = xs
    sw = sliding_window_view(xpad, PIN, axis=2)[:, :, ::PO, :]  # [B,C,NJ,79]
    xin = np.empty((PIN, C + XPAD, XC), dtype=NP_BF16)
    xin[:, 0:C, :] = (
        sw[:, :, :, ::-1].transpose(3, 1, 0, 2).astype(NP_BF16).reshape(PIN, C, XC)
    )

    # A[p, m] = w[78 - p - m] for 63 <= p + m <= 78
    idx = np.arange(PIN)[:, None] + np.arange(PO)[None, :]   # p + m
    amask = (idx >= PO - 1) & (idx <= PO + K - 2)
    aidx = np.clip(PO + K - 2 - idx, 0, K - 1)
    a_mat = np.where(amask[None], w[:, aidx], 0.0)           # [C, 79, PO]
    a_in = np.empty((PIN, C + APAD, PO), dtype=NP_BF16)
    a_in[:, 0:C, :] = a_mat.transpose(1, 0, 2).astype(NP_BF16)

    return {"xin": xin, "a_in": a_in}


def run(x, weight, bias, trace=False):
    nc = _get_nc()
    in_maps = [_prep_core(x, weight, core) for core in range(N_CORES)]
    res = run_bass_kernel_spmd(nc, in_maps, list(range(N_CORES)), trace=trace)

    y = np.empty((B, DIM, T), dtype=np.float32)
    for core in range(N_CORES):
        yp = np.asarray(res.results[core]["yout"])[:, :, 0 : C // 2, :].astype(
            np.float32
        )
        # yp[o, m, c2, (b, j)] -> y[b, 2*c2+o, 64*j + m]
        yc = yp.reshape(2, PO, C // 2, B, NJ).transpose(3, 2, 0, 4, 1)
        y[:, core * C : (core + 1) * C, :] = yc.reshape(B, C, T)
    if np.any(bias):
        y += bias[None, :, None]
    return y, res


def kernel(x, weight, bias):
    y, _ = run(
        np.asarray(x, dtype=np.float32),
        np.asarray(weight, dtype=np.float32),
        np.asarray(bias, dtype=np.float32),
    )
    return y

